# revision 2
# baseline (speedup 1.0000x reference)
"""Bilateral filter (K=7, sigma_color=0.1) on 8 Trainium2 NeuronCores.

Odd-symmetry band-layout formulation:
    If = I + S'/W
    S'  = sum_{pairs o} k_o * [u_o(p) - u_o(p-o)],   u_o = h_o * d_o
    W   = g_c + sum_{pairs o} k_o * [h_o(p) + h_o(p-o)]
    d_o(x) = I(x+o) - I(x),  h_o = (2/sqrt(pi)) exp(-d^2/sc) via ACT D_ERF,
    k_o = g_o * sqrt(pi)/2.
Each opposite tap pair shares ONE subtract, ONE activation, ONE multiply
over a slightly-expanded flat domain; the center tap vanishes from S' and
enters W as the reciprocal's bias.

Device mapping:
- 8 cores = 8 column bands of 80 cols (+3 halo each side -> 86).
- 128 partitions = 4 batches x 32 row-bands of 15 rows; each partition
  stores its 15 rows + 3-row halo as a flat [21*86]=1806 fp16 tile. All
  taps are pure flat-offset views; per-row "dead" halo columns flow
  through harmlessly and are skipped by the output DMA.
- The 7 pairs of one oy-row have consecutive offsets Delta=86*oy+ox, so
  their sub / D_ERF / mul merge into single rank-2 ops (stride-0 in0,
  stride -1 in1) -> only 4 subs + 4 ACTs + 4 muls total.
- PE accumulates +-k-scaled flat views of h (-> W) and u (-> S') into 6
  PSUM banks via scaled-identity weights; W finishes before the last
  row's S views so the reciprocal+Newton overlap the S tail.
"""
import math

import numpy as np

import concourse.bacc as bacc
import concourse.tile as tile
from concourse import mybir
from concourse.bass_utils import run_bass_kernel_spmd
import bass_rust

K = 7
PAD = K // 2
H, W = 480, 640
N = 4
NCORES = 8
SIGMA_COLOR = 2.0 * 0.1 ** 2            # 0.02
CSC = 1.0 / math.sqrt(SIGMA_COLOR)      # DErf(d*CSC) = 2/sqrt(pi)*exp(-d^2/sc)
NT = K * K

P = 128                                  # partitions (4 batches x 32 bands)
BROWS = 15                               # output rows per partition
TR = BROWS + 2 * PAD                     # 21 stored rows
COLS = W // NCORES                       # 80 output cols per core
CW = COLS + 2 * PAD                      # 86 stored cols
AL = TR * CW                             # 1806 flat image elems
PMIN = PAD * CW + PAD                    # 261: flat index of first output px
FL = (BROWS - 1) * CW + COLS             # 1284: flat output span (w/ dead)
SB = 1536                                # S'-half offset in PSUM (bank 3)

f32 = mybir.dt.float32
f16 = mybir.dt.float16

# canonical pairs grouped by oy-row: (oy, [ox...]), Delta = 86*oy + ox
_ROWS = [(0, [1, 2, 3]), (1, list(range(-3, 4))),
         (2, list(range(-3, 4))), (3, list(range(-3, 4)))]
_R2S = sorted({oy * oy + ox * ox for oy, oxs in _ROWS for ox in oxs})
NS = len(_R2S)                           # 9 radius classes

NCONV_W = 6
NCONV_S = 0

_cache = {}


def _ap(base, off, dims):
    """Rank-(1+len(dims)) AP on base's tile at element offset off."""
    return bass_rust.AP(base.tensor, base.offset + off,
                        [list(base.ap[0])] + [list(d) for d in dims])


def _act_raw(nc, out, in_, func, bias=0.0, scale=1.0):
    """Raw InstActivation (bass blocks Reciprocal in the wrapper; a Newton
    step at the call site restores accuracy)."""
    eng = nc.scalar
    inputs = [eng.lower_ap(in_)]
    for arg in (bias, scale, 0.0):
        inputs.append(mybir.ImmediateValue(dtype=mybir.dt.float32, value=arg))
    return eng.add_instruction(mybir.InstActivation(
        name=nc.get_next_instruction_name(), func=func,
        ins=inputs, outs=[eng.lower_ap(out)]))


def _build_fast(g_center):
    nc = bacc.Bacc("TRN2", target_bir_lowering=False, debug=False,
                   num_devices=NCORES)
    a_ext = nc.declare_dram_parameter("a", [P, AL], f16, isOutput=False)
    eye_ext = nc.declare_dram_parameter("eye", [P, 2 * NS, P], f16,
                                        isOutput=False)
    o_ext = nc.declare_dram_parameter("o", [P, 1200], f16, isOutput=True)

    # PSUM banked-rows layout: bank b holds output rows 5b..5b+4 as a
    # 5x86-flat span (430 fp32 of 512); W in banks 0-2, S' in banks 3-5.
    NRCH = 3
    ROWS_PER = 5
    CCOLS = ROWS_PER * COLS                       # 400 dense cols per chunk

    gspecs = [(0, [1]), (0, [2, 3]),
              (1, [-3, -2, -1, 0]), (1, [1, 2, 3]),
              (2, [-3, -2, -1, 0]), (2, [1, 2, 3]),
              (3, [-3, -2, -1, 0]), (3, [1, 2, 3])]
    groups = []
    for oy, ss in gspecs:
        dmin, dmax = 86 * oy + ss[0], 86 * oy + ss[-1]
        groups.append(dict(oy=oy, oxs=ss, np=len(ss), dmin=dmin,
                           lbar=FL + dmax))
    NPAIRS = sum(g["np"] for g in groups)

    # W-side presum conversions (flat adds on DVE; views stay banked);
    # picked from middle groups so neither PE start nor tail blocks on them
    conv_w = set()
    for g in groups[4:7]:
        for j in range(g["np"])[:2]:
            if len(conv_w) < NCONV_W:
                conv_w.add((id(g), j))
    W_LAST = 2 * NPAIRS - len(conv_w)
    S_LAST = 2 * NPAIRS

    with tile.TileContext(nc, pool_alloc_mode="queue") as tc:
        with tc.tile_pool(name="work", bufs=3) as pool, \
             tc.tile_pool(name="cst", bufs=1) as cpool, \
             tc.tile_pool(name="ep", bufs=1) as epool, \
             tc.tile_pool(name="ps", bufs=1, space="PSUM") as ppool:
            at = cpool.tile([P, AL], f16)
            eye_t = cpool.tile([P, 2 * NS, P], f16)
            nc.sync.dma_start(out=at[:, 172:1634], in_=a_ext[:, 172:1634])
            nc.sync.dma_start(out=eye_t[:, 0:NS, :], in_=eye_ext[:, 0:NS, :])
            nc.gpsimd.dma_start(out=at[:, 0:172], in_=a_ext[:, 0:172])
            nc.gpsimd.dma_start(out=at[:, 1634:AL], in_=a_ext[:, 1634:AL])
            nc.gpsimd.dma_start(out=eye_t[:, NS:2 * NS, :],
                                in_=eye_ext[:, NS:2 * NS, :])

            accW = [ppool.tile([P, 512], f32, name=f"aw{i}")
                    for i in range(3)]
            accS = [ppool.tile([P, 512], f32, name=f"as{i}")
                    for i in range(3)]

            nW = [0, 0, 0]
            nS = [0, 0, 0]

            def emit_sub(g, po=None):
                po = po or pool
                np_, lbar, dmin = g["np"], g["lbar"], g["dmin"]
                oy, ox0 = g["oy"], g["oxs"][0]
                nm = f"{oy}_{ox0}"
                dt = po.tile([P, np_, lbar], f16, name=f"d{nm}", tag="d")
                ht = po.tile([P, np_, lbar], f16, name=f"h{nm}", tag="h")
                in0 = _ap(at, PMIN, [[0, np_], [1, lbar]])
                in1 = _ap(at, PMIN - dmin, [[-1, np_], [1, lbar]])
                do = _ap(dt, 0, [[lbar, np_], [1, lbar]])
                nc.vector.tensor_tensor(do, in0, in1,
                                        mybir.AluOpType.subtract)
                nc.scalar.activation(ht[:, :, :], dt[:, :, :],
                                     mybir.ActivationFunctionType.
                                     Derivative_Erf, bias=0.0, scale=CSC)
                g["dt"], g["ht"] = dt, ht

            def emit_mul(g, po=None):
                po = po or pool
                np_, lbar = g["np"], g["lbar"]
                oy, ox0 = g["oy"], g["oxs"][0]
                dt, ht = g["dt"], g["ht"]
                ut = po.tile([P, np_, lbar], f16, name=f"u{oy}_{ox0}",
                             tag="u")
                nc.vector.tensor_tensor(ut[:, :, :], ht[:, :, :],
                                        dt[:, :, :], mybir.AluOpType.mult)
                g["ut"] = ut
                tfh = ht.rearrange("p a b -> p (a b)")
                g["hs"] = {}
                for j, ox in enumerate(g["oxs"]):
                    if (id(g), j) not in conv_w:
                        continue
                    delta = 86 * oy + ox
                    ps_t = pool.tile([P, FL], f16,
                                     name=f"c{oy}_{ox}", tag="c")
                    nc.vector.tensor_tensor(
                        ps_t[:, :],
                        _ap(tfh, j * lbar + delta, [[1, FL]]),
                        _ap(tfh, j * lbar, [[1, FL]]),
                        mybir.AluOpType.add)
                    g["hs"][j] = ps_t

            def mm(bank, slot, vt, off, ci, nn, last):
                rhs = _ap(vt, off + 86 * ROWS_PER * ci,
                          [[86, ROWS_PER], [1, COLS]])
                out = _ap(bank[ci], 0, [[86, ROWS_PER], [1, COLS]])
                nc.tensor.matmul(out, eye_t[:, slot, :], rhs,
                                 start=(nn[ci] == 0),
                                 stop=(nn[ci] == last - 1))
                nn[ci] += 1

            def side_views(g, side):
                oy, lbar = g["oy"], g["lbar"]
                t = g["ht"] if side == "W" else g["ut"]
                tf = t.rearrange("p a b -> p (a b)")
                views = []
                pres = []
                for j, ox in enumerate(g["oxs"]):
                    delta = 86 * oy + ox
                    s = _R2S.index(oy * oy + ox * ox)
                    if side == "W" and j in g["hs"]:
                        pres.append((g["hs"][j], 0, s))
                    else:
                        views.append((tf, j * lbar + delta, s))
                        views.append((tf, j * lbar,
                                      s if side == "W" else NS + s))
                return views + pres

            # epilogue tiles (dense 1200)
            r0_t = epool.tile([P, 1200], f32)
            t_t = epool.tile([P, 1200], f32)
            q_t = epool.tile([P, 1200], f32)
            x_t = epool.tile([P, 1200], f32)
            of = epool.tile([P, 1200], f16)

            def acc_ap(bank, ci):
                return _ap(bank[ci], 0, [[86, ROWS_PER], [1, COLS]])

            def dn(tile_, ci):
                return _ap(tile_, CCOLS * ci, [[COLS, ROWS_PER], [1, COLS]])

            def emit_recip():
                for ci in range(NRCH):
                    _act_raw(nc, dn(r0_t, ci), acc_ap(accW, ci),
                             mybir.ActivationFunctionType.Reciprocal,
                             bias=float(g_center))
                    nc.vector.scalar_tensor_tensor(
                        dn(t_t, ci), acc_ap(accW, ci), float(g_center),
                        dn(r0_t, ci),
                        mybir.AluOpType.add, mybir.AluOpType.mult)
                    nc.vector.scalar_tensor_tensor(
                        dn(q_t, ci), dn(t_t, ci), 2.0, dn(r0_t, ci),
                        mybir.AluOpType.subtract, mybir.AluOpType.mult)

            def emit_tail_chunk(ci):
                nc.vector.scalar_tensor_tensor(
                    dn(x_t, ci), acc_ap(accS, ci), -1.0, dn(q_t, ci),
                    mybir.AluOpType.mult, mybir.AluOpType.mult)
                iv = _ap(at, PMIN + 86 * ROWS_PER * ci,
                         [[86, ROWS_PER], [1, COLS]])
                nc.vector.tensor_tensor(
                    _ap(of, CCOLS * ci, [[COLS, ROWS_PER], [1, COLS]]),
                    dn(x_t, ci), iv, mybir.AluOpType.add)
                nc.sync.dma_start(out=o_ext[:, CCOLS * ci:CCOLS * (ci + 1)],
                                  in_=of[:, CCOLS * ci:CCOLS * (ci + 1)])

            def emit_W(g):
                for (vt, off, slot) in side_views(g, "W"):
                    for ci in range(NRCH):
                        mm(accW, slot, vt, off, ci, nW, W_LAST)

            def emit_S(g):
                for (vt, off, slot) in side_views(g, "S"):
                    for ci in range(NRCH):
                        mm(accS, slot, vt, off, ci, nS, S_LAST)

            g0, gS = groups[0], groups[1]
            rest = groups[2:]
            emit_sub(g0)
            emit_sub(gS, cpool)
            emit_mul(g0)
            emit_W(g0)
            emit_mul(gS, cpool)
            emit_W(gS)
            emit_sub(rest[0])
            prev = g0
            for i, g in enumerate(rest):
                if i + 1 < len(rest):
                    emit_sub(rest[i + 1])
                emit_mul(g)
                emit_W(g)
                emit_S(prev)
                prev = g
            emit_recip()
            emit_S(prev)
            sviews = side_views(gS, "S")
            for ci in range(NRCH):
                for (vt, off, slot) in sviews:
                    mm(accS, slot, vt, off, ci, nS, S_LAST)
                emit_tail_chunk(ci)
    nc.compile()
    return nc


def _get_nc(fast):
    assert fast, "fallback path is numpy-only"
    gc = _cache["g_center"]
    if _cache.get("fast_gc") != gc:
        _cache["fast"] = _build_fast(gc)
        _cache["fast_gc"] = gc
    return _cache["fast"]


def _shard_image(I):
    """I: (N,1,H,W) f32 -> per-core [P, AL] fp16 tiles."""
    Ip = np.zeros((N, H + 2 * PAD, W + 2 * PAD), np.float16)
    Ip[:, PAD:PAD + H, PAD:PAD + W] = I[:, 0]
    shards = []
    s0, s1 = Ip.strides[1], Ip.strides[2]
    for c in range(NCORES):
        blk = Ip[:, :, COLS * c:COLS * c + CW]       # (4, 486, 86)
        bands = np.lib.stride_tricks.as_strided(
            blk, shape=(N, 32, TR, CW),
            strides=(Ip.strides[0], BROWS * s0, s0, s1))
        shards.append(np.ascontiguousarray(bands).reshape(P, AL))
    return shards


def _eye(gdict):
    eye = np.zeros((P, 2 * NS, P), np.float32)
    idx = np.arange(P)
    for s, r2 in enumerate(_R2S):
        k = gdict[r2] * math.sqrt(math.pi) / 2.0
        eye[idx, s, idx] = k
        eye[idx, NS + s, idx] = -k
    return eye.astype(np.float16)


def _prepare(I, g):
    I = np.ascontiguousarray(np.asarray(I, dtype=np.float32))
    g = np.asarray(g, dtype=np.float32)
    gs = g[0, :, 0, 0]
    fast = bool(np.array_equal(
        g, np.broadcast_to(gs[None, :, None, None], g.shape))) and bool(
        np.all(gs > 0))
    if not fast:
        return False, None
    gdict = {}
    ok = True
    for t in range(NT):
        r2 = (t // K - PAD) ** 2 + (t % K - PAD) ** 2
        if r2 in gdict:
            ok = ok and abs(gdict[r2] - float(gs[t])) <= 1e-6 * abs(gdict[r2])
        else:
            gdict[r2] = float(gs[t])
    if not ok:
        return False, None
    _cache["g_center"] = gdict[0]
    eye = _eye(gdict)
    in_maps = [{"a": a, "eye": eye} for a in _shard_image(I)]
    return True, in_maps


def _numpy_fallback(I, g):
    I64 = np.asarray(I, np.float64)
    Ip = np.pad(I64[:, 0], ((0, 0), (PAD, PAD), (PAD, PAD)))
    out_w = np.zeros((N, H, W))
    out_s = np.zeros((N, H, W))
    g64 = np.asarray(g, np.float64)
    for t in range(NT):
        y, x = divmod(t, K)
        tap = Ip[:, y:y + H, x:x + W]
        d = tap - I64[:, 0]
        e = np.exp(-d * d / SIGMA_COLOR) * g64[:, t]
        out_w += e
        out_s += e * tap
    return (out_s / out_w).astype(np.float32)


def kernel(I, g):
    fast, in_maps = _prepare(I, g)
    if not fast:
        return _numpy_fallback(I, g)
    nc = _get_nc(True)
    res = run_bass_kernel_spmd(nc, in_maps, list(range(NCORES)))
    out = np.empty((N, H, W), np.float32)
    for c in range(NCORES):
        o = res.results[c]["o"]                      # [P, 1200] f16
        out[:, :, COLS * c:COLS * (c + 1)] = (
            o.reshape(N, H, COLS).astype(np.float32))
    return out


# revision 3
# speedup vs baseline: 1.0603x; 1.0603x over previous
"""Bilateral filter (K=7, sigma_color=0.1) on 8 Trainium2 NeuronCores.

Odd-symmetry band-layout formulation:
    If = I + S'/W
    S'  = sum_{pairs o} k_o * [u_o(p) - u_o(p-o)],   u_o = h_o * d_o
    W   = g_c + sum_{pairs o} k_o * [h_o(p) + h_o(p-o)]
    d_o(x) = I(x+o) - I(x),  h_o = (2/sqrt(pi)) exp(-d^2/sc) via ACT D_ERF,
    k_o = g_o * sqrt(pi)/2.
Each opposite tap pair shares ONE subtract, ONE activation, ONE multiply
over a slightly-expanded flat domain; the center tap vanishes from S' and
enters W as the reciprocal's bias.

Device mapping:
- 8 cores = 8 column bands of 80 cols (+3 halo each side -> 86).
- 128 partitions = 4 batches x 32 row-bands of 15 rows; each partition
  stores its 15 rows + 3-row halo as a flat [21*86]=1806 fp16 tile. All
  taps are pure flat-offset views; per-row "dead" halo columns flow
  through harmlessly and are skipped by the output DMA.
- The 7 pairs of one oy-row have consecutive offsets Delta=86*oy+ox, so
  their sub / D_ERF / mul merge into single rank-2 ops (stride-0 in0,
  stride -1 in1) -> only 4 subs + 4 ACTs + 4 muls total.
- PE accumulates +-k-scaled flat views of h (-> W) and u (-> S') into 6
  PSUM banks via scaled-identity weights; W finishes before the last
  row's S views so the reciprocal+Newton overlap the S tail.
"""
import math

import numpy as np

import concourse.bacc as bacc
import concourse.tile as tile
from concourse import mybir
from concourse.bass_utils import run_bass_kernel_spmd
import bass_rust

K = 7
PAD = K // 2
H, W = 480, 640
N = 4
NCORES = 8
SIGMA_COLOR = 2.0 * 0.1 ** 2            # 0.02
CSC = 1.0 / math.sqrt(SIGMA_COLOR)      # DErf(d*CSC) = 2/sqrt(pi)*exp(-d^2/sc)
NT = K * K

P = 128                                  # partitions (4 batches x 32 bands)
BROWS = 15                               # output rows per partition
TR = BROWS + 2 * PAD                     # 21 stored rows
COLS = W // NCORES                       # 80 output cols per core
CW = COLS + 2 * PAD                      # 86 stored cols
AL = TR * CW                             # 1806 flat image elems
PMIN = PAD * CW + PAD                    # 261: flat index of first output px
FL = (BROWS - 1) * CW + COLS             # 1284: flat output span (w/ dead)
SB = 1536                                # S'-half offset in PSUM (bank 3)

f32 = mybir.dt.float32
f16 = mybir.dt.float16

# canonical pairs grouped by oy-row: (oy, [ox...]), Delta = 86*oy + ox
_ROWS = [(0, [1, 2, 3]), (1, list(range(-3, 4))),
         (2, list(range(-3, 4))), (3, list(range(-3, 4)))]
_R2S = sorted({oy * oy + ox * ox for oy, oxs in _ROWS for ox in oxs})
NS = len(_R2S)                           # 9 radius classes

NCONV_W = 8
NCONV_S = 0

_cache = {}


def _ap(base, off, dims):
    """Rank-(1+len(dims)) AP on base's tile at element offset off."""
    return bass_rust.AP(base.tensor, base.offset + off,
                        [list(base.ap[0])] + [list(d) for d in dims])


def _act_raw(nc, out, in_, func, bias=0.0, scale=1.0):
    """Raw InstActivation (bass blocks Reciprocal in the wrapper; a Newton
    step at the call site restores accuracy)."""
    eng = nc.scalar
    inputs = [eng.lower_ap(in_)]
    for arg in (bias, scale, 0.0):
        inputs.append(mybir.ImmediateValue(dtype=mybir.dt.float32, value=arg))
    return eng.add_instruction(mybir.InstActivation(
        name=nc.get_next_instruction_name(), func=func,
        ins=inputs, outs=[eng.lower_ap(out)]))


def _build_fast(g_center):
    nc = bacc.Bacc("TRN2", target_bir_lowering=False, debug=False,
                   num_devices=NCORES)
    a_ext = nc.declare_dram_parameter("a", [P, AL], f16, isOutput=False)
    eye_ext = nc.declare_dram_parameter("eye", [P, 2 * NS, P], f16,
                                        isOutput=False)
    o_ext = nc.declare_dram_parameter("o", [P, 1200], f16, isOutput=True)

    # PSUM banked-rows layout: bank b holds output rows 5b..5b+4 as a
    # 5x86-flat span (430 fp32 of 512); W in banks 0-2, S' in banks 3-5.
    NRCH = 3
    ROWS_PER = 5
    CCOLS = ROWS_PER * COLS                       # 400 dense cols per chunk

    gspecs = [(0, [1]), (0, [2, 3]),
              (1, [-3, -2]), (1, [-1, 0]), (1, [1, 2, 3]),
              (2, [-3, -2, -1, 0]), (2, [1, 2, 3]),
              (3, [-3, -2, -1, 0]), (3, [1]), (3, [2, 3])]
    groups = []
    for oy, ss in gspecs:
        dmin, dmax = 86 * oy + ss[0], 86 * oy + ss[-1]
        groups.append(dict(oy=oy, oxs=ss, np=len(ss), dmin=dmin,
                           lbar=FL + dmax))
    NPAIRS = sum(g["np"] for g in groups)

    # W-side presum conversions (flat adds on DVE; views stay banked);
    # picked from middle groups so neither PE start nor tail blocks on them
    conv_w = set()
    for g in groups[4:8]:
        for j in list(range(g["np"]))[:2]:
            if len(conv_w) < NCONV_W:
                conv_w.add((id(g), j))
    W_LAST = 2 * NPAIRS - len(conv_w)
    S_LAST = 2 * NPAIRS

    with tile.TileContext(nc, pool_alloc_mode="queue") as tc:
        with tc.tile_pool(name="work", bufs=3) as pool, \
             tc.tile_pool(name="cst", bufs=1) as cpool, \
             tc.tile_pool(name="ep", bufs=1) as epool, \
             tc.tile_pool(name="ps", bufs=1, space="PSUM") as ppool:
            at = cpool.tile([P, AL], f16)
            eye_t = cpool.tile([P, 2 * NS, P], f16)
            nc.sync.dma_start(out=at[:, 172:1634], in_=a_ext[:, 172:1634])
            nc.sync.dma_start(out=eye_t[:, 0:NS, :], in_=eye_ext[:, 0:NS, :])
            nc.gpsimd.dma_start(out=at[:, 0:172], in_=a_ext[:, 0:172])
            nc.gpsimd.dma_start(out=at[:, 1634:AL], in_=a_ext[:, 1634:AL])
            nc.gpsimd.dma_start(out=eye_t[:, NS:2 * NS, :],
                                in_=eye_ext[:, NS:2 * NS, :])

            accW = [ppool.tile([P, 512], f32, name=f"aw{i}")
                    for i in range(3)]
            accS = [ppool.tile([P, 512], f32, name=f"as{i}")
                    for i in range(3)]

            nW = [0, 0, 0]
            nS = [0, 0, 0]

            def emit_sub(g, po=None):
                po = po or pool
                np_, lbar, dmin = g["np"], g["lbar"], g["dmin"]
                oy, ox0 = g["oy"], g["oxs"][0]
                nm = f"{oy}_{ox0}"
                dt = po.tile([P, np_, lbar], f16, name=f"d{nm}", tag="d")
                ht = po.tile([P, np_, lbar], f16, name=f"h{nm}", tag="h")
                in0 = _ap(at, PMIN, [[0, np_], [1, lbar]])
                in1 = _ap(at, PMIN - dmin, [[-1, np_], [1, lbar]])
                do = _ap(dt, 0, [[lbar, np_], [1, lbar]])
                nc.vector.tensor_tensor(do, in0, in1,
                                        mybir.AluOpType.subtract)
                nc.scalar.activation(ht[:, :, :], dt[:, :, :],
                                     mybir.ActivationFunctionType.
                                     Derivative_Erf, bias=0.0, scale=CSC)
                g["dt"], g["ht"] = dt, ht

            def emit_mul(g, po=None):
                po = po or pool
                np_, lbar = g["np"], g["lbar"]
                oy, ox0 = g["oy"], g["oxs"][0]
                dt, ht = g["dt"], g["ht"]
                ut = po.tile([P, np_, lbar], f16, name=f"u{oy}_{ox0}",
                             tag="u")
                nc.vector.tensor_tensor(ut[:, :, :], ht[:, :, :],
                                        dt[:, :, :], mybir.AluOpType.mult)
                g["ut"] = ut
                tfh = ht.rearrange("p a b -> p (a b)")
                g["hs"] = {}
                for j, ox in enumerate(g["oxs"]):
                    if (id(g), j) not in conv_w:
                        continue
                    delta = 86 * oy + ox
                    ps_t = pool.tile([P, FL], f16,
                                     name=f"c{oy}_{ox}", tag="c")
                    nc.vector.tensor_tensor(
                        ps_t[:, :],
                        _ap(tfh, j * lbar + delta, [[1, FL]]),
                        _ap(tfh, j * lbar, [[1, FL]]),
                        mybir.AluOpType.add)
                    g["hs"][j] = ps_t

            def mm(bank, slot, vt, off, ci, nn, last):
                rhs = _ap(vt, off + 86 * ROWS_PER * ci,
                          [[86, ROWS_PER], [1, COLS]])
                out = _ap(bank[ci], 0, [[86, ROWS_PER], [1, COLS]])
                nc.tensor.matmul(out, eye_t[:, slot, :], rhs,
                                 start=(nn[ci] == 0),
                                 stop=(nn[ci] == last - 1))
                nn[ci] += 1

            def side_views(g, side):
                oy, lbar = g["oy"], g["lbar"]
                t = g["ht"] if side == "W" else g["ut"]
                tf = t.rearrange("p a b -> p (a b)")
                views = []
                pres = []
                for j, ox in enumerate(g["oxs"]):
                    delta = 86 * oy + ox
                    s = _R2S.index(oy * oy + ox * ox)
                    if side == "W" and j in g["hs"]:
                        pres.append((g["hs"][j], 0, s))
                    else:
                        views.append((tf, j * lbar + delta, s))
                        views.append((tf, j * lbar,
                                      s if side == "W" else NS + s))
                return views + pres

            # epilogue tiles (dense 1200)
            r0_t = epool.tile([P, 1200], f32)
            t_t = epool.tile([P, 1200], f32)
            q_t = epool.tile([P, 1200], f32)
            x_t = epool.tile([P, 1200], f32)
            of = epool.tile([P, 1200], f16)

            def acc_ap(bank, ci):
                return _ap(bank[ci], 0, [[86, ROWS_PER], [1, COLS]])

            def dn(tile_, ci):
                return _ap(tile_, CCOLS * ci, [[COLS, ROWS_PER], [1, COLS]])

            def emit_recip():
                for ci in range(NRCH):
                    _act_raw(nc, dn(r0_t, ci), acc_ap(accW, ci),
                             mybir.ActivationFunctionType.Reciprocal,
                             bias=float(g_center))
                    nc.vector.scalar_tensor_tensor(
                        dn(t_t, ci), acc_ap(accW, ci), float(g_center),
                        dn(r0_t, ci),
                        mybir.AluOpType.add, mybir.AluOpType.mult)
                    nc.vector.scalar_tensor_tensor(
                        dn(q_t, ci), dn(t_t, ci), 2.0, dn(r0_t, ci),
                        mybir.AluOpType.subtract, mybir.AluOpType.mult)

            def emit_tail_chunk(ci):
                # final chunk split in two so the last x/add/dma cascade is
                # half as deep
                parts = ((0, ROWS_PER),) if ci < NRCH - 1 else \
                    ((0, 3), (3, ROWS_PER))
                for (r0, r1) in parts:
                    nr = r1 - r0
                    def sl(t):
                        return _ap(t, CCOLS * ci + COLS * r0,
                                   [[COLS, nr], [1, COLS]])
                    nc.vector.scalar_tensor_tensor(
                        sl(x_t),
                        _ap(accS[ci], 86 * r0, [[86, nr], [1, COLS]]),
                        -1.0, sl(q_t),
                        mybir.AluOpType.mult, mybir.AluOpType.mult)
                    iv = _ap(at, PMIN + 86 * (ROWS_PER * ci + r0),
                             [[86, nr], [1, COLS]])
                    nc.vector.tensor_tensor(sl(of), sl(x_t), iv,
                                            mybir.AluOpType.add)
                    c0 = CCOLS * ci + COLS * r0
                    c1 = CCOLS * ci + COLS * r1
                    nc.sync.dma_start(out=o_ext[:, c0:c1],
                                      in_=of[:, c0:c1])

            def emit_W(g):
                for (vt, off, slot) in side_views(g, "W"):
                    for ci in range(NRCH):
                        mm(accW, slot, vt, off, ci, nW, W_LAST)

            def emit_S(g):
                for (vt, off, slot) in side_views(g, "S"):
                    for ci in range(NRCH):
                        mm(accS, slot, vt, off, ci, nS, S_LAST)

            g0, gS = groups[0], groups[1]
            rest = groups[2:]
            emit_sub(g0)
            emit_sub(gS, cpool)
            emit_mul(g0)
            emit_W(g0)
            emit_mul(gS, cpool)
            emit_W(gS)
            emit_sub(rest[0])
            for i, g in enumerate(rest):
                if i + 1 < len(rest):
                    emit_sub(rest[i + 1])
                emit_mul(g)
                emit_W(g)
                if i == len(rest) - 1:
                    emit_recip()
                emit_S(g)
                if i == 0:
                    emit_S(g0)
            sviews = side_views(gS, "S")
            for ci in range(NRCH):
                for (vt, off, slot) in sviews:
                    mm(accS, slot, vt, off, ci, nS, S_LAST)
                emit_tail_chunk(ci)
    nc.compile()
    return nc


def _get_nc(fast):
    assert fast, "fallback path is numpy-only"
    gc = _cache["g_center"]
    if _cache.get("fast_gc") != gc:
        _cache["fast"] = _build_fast(gc)
        _cache["fast_gc"] = gc
    return _cache["fast"]


def _shard_image(I):
    """I: (N,1,H,W) f32 -> per-core [P, AL] fp16 tiles."""
    Ip = np.zeros((N, H + 2 * PAD, W + 2 * PAD), np.float16)
    Ip[:, PAD:PAD + H, PAD:PAD + W] = I[:, 0]
    shards = []
    s0, s1 = Ip.strides[1], Ip.strides[2]
    for c in range(NCORES):
        blk = Ip[:, :, COLS * c:COLS * c + CW]       # (4, 486, 86)
        bands = np.lib.stride_tricks.as_strided(
            blk, shape=(N, 32, TR, CW),
            strides=(Ip.strides[0], BROWS * s0, s0, s1))
        shards.append(np.ascontiguousarray(bands).reshape(P, AL))
    return shards


def _eye(gdict):
    eye = np.zeros((P, 2 * NS, P), np.float32)
    idx = np.arange(P)
    for s, r2 in enumerate(_R2S):
        k = gdict[r2] * math.sqrt(math.pi) / 2.0
        eye[idx, s, idx] = k
        eye[idx, NS + s, idx] = -k
    return eye.astype(np.float16)


def _prepare(I, g):
    I = np.ascontiguousarray(np.asarray(I, dtype=np.float32))
    g = np.asarray(g, dtype=np.float32)
    gs = g[0, :, 0, 0]
    fast = bool(np.array_equal(
        g, np.broadcast_to(gs[None, :, None, None], g.shape))) and bool(
        np.all(gs > 0))
    if not fast:
        return False, None
    gdict = {}
    ok = True
    for t in range(NT):
        r2 = (t // K - PAD) ** 2 + (t % K - PAD) ** 2
        if r2 in gdict:
            ok = ok and abs(gdict[r2] - float(gs[t])) <= 1e-6 * abs(gdict[r2])
        else:
            gdict[r2] = float(gs[t])
    if not ok:
        return False, None
    _cache["g_center"] = gdict[0]
    eye = _eye(gdict)
    in_maps = [{"a": a, "eye": eye} for a in _shard_image(I)]
    return True, in_maps


def _numpy_fallback(I, g):
    I64 = np.asarray(I, np.float64)
    Ip = np.pad(I64[:, 0], ((0, 0), (PAD, PAD), (PAD, PAD)))
    out_w = np.zeros((N, H, W))
    out_s = np.zeros((N, H, W))
    g64 = np.asarray(g, np.float64)
    for t in range(NT):
        y, x = divmod(t, K)
        tap = Ip[:, y:y + H, x:x + W]
        d = tap - I64[:, 0]
        e = np.exp(-d * d / SIGMA_COLOR) * g64[:, t]
        out_w += e
        out_s += e * tap
    return (out_s / out_w).astype(np.float32)


def kernel(I, g):
    fast, in_maps = _prepare(I, g)
    if not fast:
        return _numpy_fallback(I, g)
    nc = _get_nc(True)
    res = run_bass_kernel_spmd(nc, in_maps, list(range(NCORES)))
    out = np.empty((N, H, W), np.float32)
    for c in range(NCORES):
        o = res.results[c]["o"]                      # [P, 1200] f16
        out[:, :, COLS * c:COLS * (c + 1)] = (
            o.reshape(N, H, COLS).astype(np.float32))
    return out


# revision 4
# speedup vs baseline: 1.0607x; 1.0004x over previous
"""Bilateral filter (K=7, sigma_color=0.1) on 8 Trainium2 NeuronCores.

Odd-symmetry band-layout formulation:
    If = I + S'/W
    S'  = sum_{pairs o} k_o * [u_o(p) - u_o(p-o)],   u_o = h_o * d_o
    W   = g_c + sum_{pairs o} k_o * [h_o(p) + h_o(p-o)]
    d_o(x) = I(x+o) - I(x),  h_o = (2/sqrt(pi)) exp(-d^2/sc) via ACT D_ERF,
    k_o = g_o * sqrt(pi)/2.
Each opposite tap pair shares ONE subtract, ONE activation, ONE multiply
over a slightly-expanded flat domain; the center tap vanishes from S' and
enters W as the reciprocal's bias.

Device mapping:
- 8 cores = 8 column bands of 80 cols (+3 halo each side -> 86).
- 128 partitions = 4 batches x 32 row-bands of 15 rows; each partition
  stores its 15 rows + 3-row halo as a flat [21*86]=1806 fp16 tile. All
  taps are pure flat-offset views; per-row "dead" halo columns flow
  through harmlessly and are skipped by the output DMA.
- The 7 pairs of one oy-row have consecutive offsets Delta=86*oy+ox, so
  their sub / D_ERF / mul merge into single rank-2 ops (stride-0 in0,
  stride -1 in1) -> only 4 subs + 4 ACTs + 4 muls total.
- PE accumulates +-k-scaled flat views of h (-> W) and u (-> S') into 6
  PSUM banks via scaled-identity weights; W finishes before the last
  row's S views so the reciprocal+Newton overlap the S tail.
"""
import math

import numpy as np

import concourse.bacc as bacc
import concourse.tile as tile
from concourse import mybir
from concourse.bass_utils import run_bass_kernel_spmd
import bass_rust

K = 7
PAD = K // 2
H, W = 480, 640
N = 4
NCORES = 8
SIGMA_COLOR = 2.0 * 0.1 ** 2            # 0.02
CSC = 1.0 / math.sqrt(SIGMA_COLOR)      # DErf(d*CSC) = 2/sqrt(pi)*exp(-d^2/sc)
NT = K * K

P = 128                                  # partitions (4 batches x 32 bands)
BROWS = 15                               # output rows per partition
TR = BROWS + 2 * PAD                     # 21 stored rows
COLS = W // NCORES                       # 80 output cols per core
CW = COLS + 2 * PAD                      # 86 stored cols
AL = TR * CW                             # 1806 flat image elems
PMIN = PAD * CW + PAD                    # 261: flat index of first output px
FL = (BROWS - 1) * CW + COLS             # 1284: flat output span (w/ dead)
SB = 1536                                # S'-half offset in PSUM (bank 3)

f32 = mybir.dt.float32
f16 = mybir.dt.float16

# canonical pairs grouped by oy-row: (oy, [ox...]), Delta = 86*oy + ox
_ROWS = [(0, [1, 2, 3]), (1, list(range(-3, 4))),
         (2, list(range(-3, 4))), (3, list(range(-3, 4)))]
_R2S = sorted({oy * oy + ox * ox for oy, oxs in _ROWS for ox in oxs})
NS = len(_R2S)                           # 9 radius classes

NCONV_W = 8
NCONV_S = 0

_cache = {}


def _ap(base, off, dims):
    """Rank-(1+len(dims)) AP on base's tile at element offset off."""
    return bass_rust.AP(base.tensor, base.offset + off,
                        [list(base.ap[0])] + [list(d) for d in dims])


def _act_raw(nc, out, in_, func, bias=0.0, scale=1.0):
    """Raw InstActivation (bass blocks Reciprocal in the wrapper; a Newton
    step at the call site restores accuracy)."""
    eng = nc.scalar
    inputs = [eng.lower_ap(in_)]
    for arg in (bias, scale, 0.0):
        inputs.append(mybir.ImmediateValue(dtype=mybir.dt.float32, value=arg))
    return eng.add_instruction(mybir.InstActivation(
        name=nc.get_next_instruction_name(), func=func,
        ins=inputs, outs=[eng.lower_ap(out)]))


def _build_fast(g_center):
    nc = bacc.Bacc("TRN2", target_bir_lowering=False, debug=False,
                   num_devices=NCORES)
    a_ext = nc.declare_dram_parameter("a", [P, AL], f16, isOutput=False)
    eye_ext = nc.declare_dram_parameter("eye", [P, 2 * NS, P], f16,
                                        isOutput=False)
    o_ext = nc.declare_dram_parameter("o", [P, 1200], f16, isOutput=True)

    # PSUM banked-rows layout: bank b holds output rows 5b..5b+4 as a
    # 5x86-flat span (430 fp32 of 512); W in banks 0-2, S' in banks 3-5.
    NRCH = 3
    ROWS_PER = 5
    CCOLS = ROWS_PER * COLS                       # 400 dense cols per chunk

    gspecs = [(0, [1]), (0, [2, 3]),
              (1, [-3, -2]), (1, [-1, 0]), (1, [1, 2, 3]),
              (2, [-3, -2, -1, 0]), (2, [1, 2, 3]),
              (3, [-3, -2, -1, 0]), (3, [1]), (3, [2, 3])]
    groups = []
    for oy, ss in gspecs:
        dmin, dmax = 86 * oy + ss[0], 86 * oy + ss[-1]
        groups.append(dict(oy=oy, oxs=ss, np=len(ss), dmin=dmin,
                           lbar=FL + dmax))
    NPAIRS = sum(g["np"] for g in groups)

    # W-side presum conversions (flat adds on DVE; views stay banked);
    # picked from middle groups so neither PE start nor tail blocks on them
    conv_w = set()
    for g in groups[4:8]:
        for j in list(range(g["np"]))[:2]:
            if len(conv_w) < NCONV_W:
                conv_w.add((id(g), j))
    W_LAST = 2 * NPAIRS - len(conv_w)
    S_LAST = 2 * NPAIRS

    with tile.TileContext(nc, pool_alloc_mode="queue") as tc:
        with tc.tile_pool(name="work", bufs=3) as pool, \
             tc.tile_pool(name="cst", bufs=1) as cpool, \
             tc.tile_pool(name="ep", bufs=1) as epool, \
             tc.tile_pool(name="ps", bufs=1, space="PSUM") as ppool:
            at = cpool.tile([P, AL], f16)
            eye_t = cpool.tile([P, 2 * NS, P], f16)
            nc.sync.dma_start(out=at[:, 172:1634], in_=a_ext[:, 172:1634])
            nc.gpsimd.dma_start(out=at[:, 0:172], in_=a_ext[:, 0:172])
            nc.gpsimd.dma_start(out=at[:, 1634:AL], in_=a_ext[:, 1634:AL])
            nc.sync.dma_start(out=eye_t[:, 0:NS, :], in_=eye_ext[:, 0:NS, :])
            nc.gpsimd.dma_start(out=eye_t[:, NS:2 * NS, :],
                                in_=eye_ext[:, NS:2 * NS, :])

            accW = [ppool.tile([P, 512], f32, name=f"aw{i}")
                    for i in range(3)]
            accS = [ppool.tile([P, 512], f32, name=f"as{i}")
                    for i in range(3)]

            nW = [0, 0, 0]
            nS = [0, 0, 0]

            def emit_sub(g, po=None):
                po = po or pool
                np_, lbar, dmin = g["np"], g["lbar"], g["dmin"]
                oy, ox0 = g["oy"], g["oxs"][0]
                nm = f"{oy}_{ox0}"
                dt = po.tile([P, np_, lbar], f16, name=f"d{nm}", tag="d")
                ht = po.tile([P, np_, lbar], f16, name=f"h{nm}", tag="h")
                in0 = _ap(at, PMIN, [[0, np_], [1, lbar]])
                in1 = _ap(at, PMIN - dmin, [[-1, np_], [1, lbar]])
                do = _ap(dt, 0, [[lbar, np_], [1, lbar]])
                nc.vector.tensor_tensor(do, in0, in1,
                                        mybir.AluOpType.subtract)
                nc.scalar.activation(ht[:, :, :], dt[:, :, :],
                                     mybir.ActivationFunctionType.
                                     Derivative_Erf, bias=0.0, scale=CSC)
                g["dt"], g["ht"] = dt, ht

            def emit_mul(g, po=None):
                po = po or pool
                np_, lbar = g["np"], g["lbar"]
                oy, ox0 = g["oy"], g["oxs"][0]
                dt, ht = g["dt"], g["ht"]
                ut = po.tile([P, np_, lbar], f16, name=f"u{oy}_{ox0}",
                             tag="u")
                nc.vector.tensor_tensor(ut[:, :, :], ht[:, :, :],
                                        dt[:, :, :], mybir.AluOpType.mult)
                g["ut"] = ut
                tfh = ht.rearrange("p a b -> p (a b)")
                g["hs"] = {}
                for j, ox in enumerate(g["oxs"]):
                    if (id(g), j) not in conv_w:
                        continue
                    delta = 86 * oy + ox
                    ps_t = pool.tile([P, FL], f16,
                                     name=f"c{oy}_{ox}", tag="c")
                    nc.vector.tensor_tensor(
                        ps_t[:, :],
                        _ap(tfh, j * lbar + delta, [[1, FL]]),
                        _ap(tfh, j * lbar, [[1, FL]]),
                        mybir.AluOpType.add)
                    g["hs"][j] = ps_t

            def mm(bank, slot, vt, off, ci, nn, last):
                rhs = _ap(vt, off + 86 * ROWS_PER * ci,
                          [[86, ROWS_PER], [1, COLS]])
                out = _ap(bank[ci], 0, [[86, ROWS_PER], [1, COLS]])
                nc.tensor.matmul(out, eye_t[:, slot, :], rhs,
                                 start=(nn[ci] == 0),
                                 stop=(nn[ci] == last - 1))
                nn[ci] += 1

            def side_views(g, side):
                oy, lbar = g["oy"], g["lbar"]
                t = g["ht"] if side == "W" else g["ut"]
                tf = t.rearrange("p a b -> p (a b)")
                views = []
                pres = []
                for j, ox in enumerate(g["oxs"]):
                    delta = 86 * oy + ox
                    s = _R2S.index(oy * oy + ox * ox)
                    if side == "W" and j in g["hs"]:
                        pres.append((g["hs"][j], 0, s))
                    else:
                        views.append((tf, j * lbar + delta, s))
                        views.append((tf, j * lbar,
                                      s if side == "W" else NS + s))
                return views + pres

            # epilogue tiles (dense 1200)
            r0_t = epool.tile([P, 1200], f32)
            t_t = epool.tile([P, 1200], f32)
            q_t = epool.tile([P, 1200], f32)
            x_t = epool.tile([P, 1200], f32)
            of = epool.tile([P, 1200], f16)

            def acc_ap(bank, ci):
                return _ap(bank[ci], 0, [[86, ROWS_PER], [1, COLS]])

            def dn(tile_, ci):
                return _ap(tile_, CCOLS * ci, [[COLS, ROWS_PER], [1, COLS]])

            def emit_recip():
                for ci in range(NRCH):
                    _act_raw(nc, dn(r0_t, ci), acc_ap(accW, ci),
                             mybir.ActivationFunctionType.Reciprocal,
                             bias=float(g_center))
                    nc.vector.scalar_tensor_tensor(
                        dn(t_t, ci), acc_ap(accW, ci), float(g_center),
                        dn(r0_t, ci),
                        mybir.AluOpType.add, mybir.AluOpType.mult)
                    nc.vector.scalar_tensor_tensor(
                        dn(q_t, ci), dn(t_t, ci), 2.0, dn(r0_t, ci),
                        mybir.AluOpType.subtract, mybir.AluOpType.mult)

            def emit_tail_chunk(ci):
                # final chunk split in two so the last x/add/dma cascade is
                # half as deep
                parts = ((0, ROWS_PER),) if ci < NRCH - 1 else \
                    ((0, 3), (3, ROWS_PER))
                for (r0, r1) in parts:
                    nr = r1 - r0
                    def sl(t):
                        return _ap(t, CCOLS * ci + COLS * r0,
                                   [[COLS, nr], [1, COLS]])
                    nc.vector.scalar_tensor_tensor(
                        sl(x_t),
                        _ap(accS[ci], 86 * r0, [[86, nr], [1, COLS]]),
                        -1.0, sl(q_t),
                        mybir.AluOpType.mult, mybir.AluOpType.mult)
                    iv = _ap(at, PMIN + 86 * (ROWS_PER * ci + r0),
                             [[86, nr], [1, COLS]])
                    nc.vector.tensor_tensor(sl(of), sl(x_t), iv,
                                            mybir.AluOpType.add)
                    c0 = CCOLS * ci + COLS * r0
                    c1 = CCOLS * ci + COLS * r1
                    nc.sync.dma_start(out=o_ext[:, c0:c1],
                                      in_=of[:, c0:c1])

            def emit_W(g):
                for (vt, off, slot) in side_views(g, "W"):
                    for ci in range(NRCH):
                        mm(accW, slot, vt, off, ci, nW, W_LAST)

            def emit_S(g):
                for (vt, off, slot) in side_views(g, "S"):
                    for ci in range(NRCH):
                        mm(accS, slot, vt, off, ci, nS, S_LAST)

            g0, gS = groups[0], groups[1]
            rest = groups[2:]
            emit_sub(g0)
            emit_sub(gS, cpool)
            emit_mul(g0)
            emit_W(g0)
            emit_mul(gS, cpool)
            emit_W(gS)
            emit_sub(rest[0])
            for i, g in enumerate(rest):
                if i + 1 < len(rest):
                    emit_sub(rest[i + 1])
                emit_mul(g)
                emit_W(g)
                if i == len(rest) - 1:
                    emit_recip()
                emit_S(g)
                if i == 0:
                    emit_S(g0)
            sviews = side_views(gS, "S")
            for ci in range(NRCH):
                for (vt, off, slot) in sviews:
                    mm(accS, slot, vt, off, ci, nS, S_LAST)
                emit_tail_chunk(ci)
    nc.compile()
    return nc


def _get_nc(fast):
    assert fast, "fallback path is numpy-only"
    gc = _cache["g_center"]
    if _cache.get("fast_gc") != gc:
        _cache["fast"] = _build_fast(gc)
        _cache["fast_gc"] = gc
    return _cache["fast"]


def _shard_image(I):
    """I: (N,1,H,W) f32 -> per-core [P, AL] fp16 tiles."""
    Ip = np.zeros((N, H + 2 * PAD, W + 2 * PAD), np.float16)
    Ip[:, PAD:PAD + H, PAD:PAD + W] = I[:, 0]
    shards = []
    s0, s1 = Ip.strides[1], Ip.strides[2]
    for c in range(NCORES):
        blk = Ip[:, :, COLS * c:COLS * c + CW]       # (4, 486, 86)
        bands = np.lib.stride_tricks.as_strided(
            blk, shape=(N, 32, TR, CW),
            strides=(Ip.strides[0], BROWS * s0, s0, s1))
        shards.append(np.ascontiguousarray(bands).reshape(P, AL))
    return shards


def _eye(gdict):
    eye = np.zeros((P, 2 * NS, P), np.float32)
    idx = np.arange(P)
    for s, r2 in enumerate(_R2S):
        k = gdict[r2] * math.sqrt(math.pi) / 2.0
        eye[idx, s, idx] = k
        eye[idx, NS + s, idx] = -k
    return eye.astype(np.float16)


def _prepare(I, g):
    I = np.ascontiguousarray(np.asarray(I, dtype=np.float32))
    g = np.asarray(g, dtype=np.float32)
    gs = g[0, :, 0, 0]
    fast = bool(np.array_equal(
        g, np.broadcast_to(gs[None, :, None, None], g.shape))) and bool(
        np.all(gs > 0))
    if not fast:
        return False, None
    gdict = {}
    ok = True
    for t in range(NT):
        r2 = (t // K - PAD) ** 2 + (t % K - PAD) ** 2
        if r2 in gdict:
            ok = ok and abs(gdict[r2] - float(gs[t])) <= 1e-6 * abs(gdict[r2])
        else:
            gdict[r2] = float(gs[t])
    if not ok:
        return False, None
    _cache["g_center"] = gdict[0]
    eye = _eye(gdict)
    in_maps = [{"a": a, "eye": eye} for a in _shard_image(I)]
    return True, in_maps


def _numpy_fallback(I, g):
    I64 = np.asarray(I, np.float64)
    Ip = np.pad(I64[:, 0], ((0, 0), (PAD, PAD), (PAD, PAD)))
    out_w = np.zeros((N, H, W))
    out_s = np.zeros((N, H, W))
    g64 = np.asarray(g, np.float64)
    for t in range(NT):
        y, x = divmod(t, K)
        tap = Ip[:, y:y + H, x:x + W]
        d = tap - I64[:, 0]
        e = np.exp(-d * d / SIGMA_COLOR) * g64[:, t]
        out_w += e
        out_s += e * tap
    return (out_s / out_w).astype(np.float32)


def kernel(I, g):
    fast, in_maps = _prepare(I, g)
    if not fast:
        return _numpy_fallback(I, g)
    nc = _get_nc(True)
    try:
        res = run_bass_kernel_spmd(nc, in_maps, list(range(NCORES)))
    except Exception:
        res = run_bass_kernel_spmd(nc, in_maps, list(range(NCORES)))
    out = np.empty((N, H, W), np.float32)
    for c in range(NCORES):
        o = res.results[c]["o"]                      # [P, 1200] f16
        out[:, :, COLS * c:COLS * (c + 1)] = (
            o.reshape(N, H, COLS).astype(np.float32))
    return out


# revision 5
# speedup vs baseline: 1.0768x; 1.0152x over previous
"""Bilateral filter (K=7, sigma_color=0.1) on 8 Trainium2 NeuronCores.

Odd-symmetry band-layout formulation:
    If = I + S'/W
    S'  = sum_{pairs o} k_o * [u_o(p) - u_o(p-o)],   u_o = h_o * d_o
    W   = g_c + sum_{pairs o} k_o * [h_o(p) + h_o(p-o)]
    d_o(x) = I(x+o) - I(x),  h_o = (2/sqrt(pi)) exp(-d^2/sc) via ACT D_ERF,
    k_o = g_o * sqrt(pi)/2.
Each opposite tap pair shares ONE subtract, ONE activation, ONE multiply
over a slightly-expanded flat domain; the center tap vanishes from S' and
enters W as the reciprocal's bias.

Device mapping:
- 8 cores = 8 column bands of 80 cols (+3 halo each side -> 86).
- 128 partitions = 4 batches x 32 row-bands of 15 rows; each partition
  stores its 15 rows + 3-row halo as a flat [21*86]=1806 fp16 tile. All
  taps are pure flat-offset views; per-row "dead" halo columns flow
  through harmlessly and are skipped by the output DMA.
- Pairs with consecutive offsets Delta=86*oy+ox merge into single
  rank-2 sub / D_ERF / mul ops (stride-0 in0, stride -1 in1); ~10
  groups, sized small at the ends so the PE pipeline ramps fast and
  drains short.
- PE accumulates +-k-scaled views of h (-> W) and u (-> S') with
  scaled-identity weights into 6 SEPARATE single-bank PSUM tiles
  (banked-rows layout: 5 output rows per bank, dense 80-col stream) --
  separate tiles keep the epilogue's PSUM reads from serializing later
  matmuls. A few pair-sums of h are precombined on DVE (NCONV_W) to
  shave PE columns. W finishes before the last S views so the
  reciprocal+Newton overlap the S tail; the output leaves as fp16.
"""
import math

import numpy as np

import concourse.bacc as bacc
import concourse.tile as tile
from concourse import mybir
from concourse.bass_utils import run_bass_kernel_spmd
import bass_rust

K = 7
PAD = K // 2
H, W = 480, 640
N = 4
NCORES = 8
SIGMA_COLOR = 2.0 * 0.1 ** 2            # 0.02
CSC = 1.0 / math.sqrt(SIGMA_COLOR)      # DErf(d*CSC) = 2/sqrt(pi)*exp(-d^2/sc)
NT = K * K

P = 128                                  # partitions (4 batches x 32 bands)
BROWS = 15                               # output rows per partition
TR = BROWS + 2 * PAD                     # 21 stored rows
COLS = W // NCORES                       # 80 output cols per core
CW = COLS + 2 * PAD                      # 86 stored cols
AL = TR * CW                             # 1806 flat image elems
PMIN = PAD * CW + PAD                    # 261: flat index of first output px
FL = (BROWS - 1) * CW + COLS             # 1284: flat output span (w/ dead)
SB = 1536                                # S'-half offset in PSUM (bank 3)

f32 = mybir.dt.float32
f16 = mybir.dt.float16

# canonical pairs grouped by oy-row: (oy, [ox...]), Delta = 86*oy + ox
_ROWS = [(0, [1, 2, 3]), (1, list(range(-3, 4))),
         (2, list(range(-3, 4))), (3, list(range(-3, 4)))]
_R2S = sorted({oy * oy + ox * ox for oy, oxs in _ROWS for ox in oxs})
NS = len(_R2S)                           # 9 radius classes

NCONV_W = 10
NCONV_S = 0

_cache = {}


def _ap(base, off, dims):
    """Rank-(1+len(dims)) AP on base's tile at element offset off."""
    return bass_rust.AP(base.tensor, base.offset + off,
                        [list(base.ap[0])] + [list(d) for d in dims])


def _act_raw(nc, out, in_, func, bias=0.0, scale=1.0):
    """Raw InstActivation (bass blocks Reciprocal in the wrapper; a Newton
    step at the call site restores accuracy)."""
    eng = nc.scalar
    inputs = [eng.lower_ap(in_)]
    for arg in (bias, scale, 0.0):
        inputs.append(mybir.ImmediateValue(dtype=mybir.dt.float32, value=arg))
    return eng.add_instruction(mybir.InstActivation(
        name=nc.get_next_instruction_name(), func=func,
        ins=inputs, outs=[eng.lower_ap(out)]))


def _build_fast(g_center):
    nc = bacc.Bacc("TRN2", target_bir_lowering=False, debug=False,
                   num_devices=NCORES)
    a_ext = nc.declare_dram_parameter("a", [P, AL], f16, isOutput=False)
    eye_ext = nc.declare_dram_parameter("eye", [P, 2 * NS, P], f16,
                                        isOutput=False)
    o_ext = nc.declare_dram_parameter("o", [P, 1200], f16, isOutput=True)

    # PSUM banked-rows layout: bank b holds output rows 5b..5b+4 as a
    # 5x86-flat span (430 fp32 of 512); W in banks 0-2, S' in banks 3-5.
    NRCH = 3
    ROWS_PER = 5
    CCOLS = ROWS_PER * COLS                       # 400 dense cols per chunk

    gspecs = [(0, [1]), (0, [2, 3]),
              (1, [-3, -2]), (1, [-1, 0]), (1, [1, 2, 3]),
              (2, [-3, -2, -1, 0]), (2, [1, 2, 3]),
              (3, [-3, -2, -1, 0]), (3, [1]), (3, [2, 3])]
    groups = []
    for oy, ss in gspecs:
        dmin, dmax = 86 * oy + ss[0], 86 * oy + ss[-1]
        groups.append(dict(oy=oy, oxs=ss, np=len(ss), dmin=dmin,
                           lbar=FL + dmax))
    NPAIRS = sum(g["np"] for g in groups)

    # W-side presum conversions (flat adds on DVE; views stay banked);
    # picked from middle groups so neither PE start nor tail blocks on them
    conv_w = set()
    for g in groups[4:9]:
        for j in list(range(g["np"]))[:2]:
            if len(conv_w) < NCONV_W:
                conv_w.add((id(g), j))
    W_LAST = 2 * NPAIRS - len(conv_w)
    S_LAST = 2 * NPAIRS

    with tile.TileContext(nc, pool_alloc_mode="queue") as tc:
        with tc.tile_pool(name="work", bufs=3) as pool, \
             tc.tile_pool(name="cst", bufs=1) as cpool, \
             tc.tile_pool(name="ep", bufs=1) as epool, \
             tc.tile_pool(name="ps", bufs=1, space="PSUM") as ppool:
            at = cpool.tile([P, AL], f16)
            eye_t = cpool.tile([P, 2 * NS, P], f16)
            nc.sync.dma_start(out=at[:, 172:1634], in_=a_ext[:, 172:1634])
            nc.gpsimd.dma_start(out=at[:, 0:172], in_=a_ext[:, 0:172])
            nc.gpsimd.dma_start(out=at[:, 1634:AL], in_=a_ext[:, 1634:AL])
            nc.sync.dma_start(out=eye_t[:, 0:NS, :], in_=eye_ext[:, 0:NS, :])
            nc.gpsimd.dma_start(out=eye_t[:, NS:2 * NS, :],
                                in_=eye_ext[:, NS:2 * NS, :])

            accW = [ppool.tile([P, 512], f32, name=f"aw{i}")
                    for i in range(3)]
            accS = [ppool.tile([P, 512], f32, name=f"as{i}")
                    for i in range(3)]

            nW = [0, 0, 0]
            nS = [0, 0, 0]

            def emit_sub(g, po=None):
                po = po or pool
                np_, lbar, dmin = g["np"], g["lbar"], g["dmin"]
                oy, ox0 = g["oy"], g["oxs"][0]
                nm = f"{oy}_{ox0}"
                dt = po.tile([P, np_, lbar], f16, name=f"d{nm}", tag="d")
                ht = po.tile([P, np_, lbar], f16, name=f"h{nm}", tag="h")
                in0 = _ap(at, PMIN, [[0, np_], [1, lbar]])
                in1 = _ap(at, PMIN - dmin, [[-1, np_], [1, lbar]])
                do = _ap(dt, 0, [[lbar, np_], [1, lbar]])
                nc.vector.tensor_tensor(do, in0, in1,
                                        mybir.AluOpType.subtract)
                nc.scalar.activation(ht[:, :, :], dt[:, :, :],
                                     mybir.ActivationFunctionType.
                                     Derivative_Erf, bias=0.0, scale=CSC)
                g["dt"], g["ht"] = dt, ht

            def emit_mul(g, po=None):
                po = po or pool
                np_, lbar = g["np"], g["lbar"]
                oy, ox0 = g["oy"], g["oxs"][0]
                dt, ht = g["dt"], g["ht"]
                ut = po.tile([P, np_, lbar], f16, name=f"u{oy}_{ox0}",
                             tag="u")
                nc.vector.tensor_tensor(ut[:, :, :], ht[:, :, :],
                                        dt[:, :, :], mybir.AluOpType.mult)
                g["ut"] = ut
                tfh = ht.rearrange("p a b -> p (a b)")
                g["hs"] = {}
                for j, ox in enumerate(g["oxs"]):
                    if (id(g), j) not in conv_w:
                        continue
                    delta = 86 * oy + ox
                    ps_t = pool.tile([P, FL], f16,
                                     name=f"c{oy}_{ox}", tag="c")
                    nc.vector.tensor_tensor(
                        ps_t[:, :],
                        _ap(tfh, j * lbar + delta, [[1, FL]]),
                        _ap(tfh, j * lbar, [[1, FL]]),
                        mybir.AluOpType.add)
                    g["hs"][j] = ps_t

            def mm(bank, slot, vt, off, ci, nn, last):
                rhs = _ap(vt, off + 86 * ROWS_PER * ci,
                          [[86, ROWS_PER], [1, COLS]])
                out = _ap(bank[ci], 0, [[86, ROWS_PER], [1, COLS]])
                nc.tensor.matmul(out, eye_t[:, slot, :], rhs,
                                 start=(nn[ci] == 0),
                                 stop=(nn[ci] == last - 1))
                nn[ci] += 1

            def side_views(g, side):
                oy, lbar = g["oy"], g["lbar"]
                t = g["ht"] if side == "W" else g["ut"]
                tf = t.rearrange("p a b -> p (a b)")
                views = []
                pres = []
                for j, ox in enumerate(g["oxs"]):
                    delta = 86 * oy + ox
                    s = _R2S.index(oy * oy + ox * ox)
                    if side == "W" and j in g["hs"]:
                        pres.append((g["hs"][j], 0, s))
                    else:
                        views.append((tf, j * lbar + delta, s))
                        views.append((tf, j * lbar,
                                      s if side == "W" else NS + s))
                return views + pres

            # epilogue tiles (dense 1200)
            r0_t = epool.tile([P, 1200], f32)
            t_t = epool.tile([P, 1200], f32)
            q_t = epool.tile([P, 1200], f32)
            x_t = epool.tile([P, 1200], f32)
            of = epool.tile([P, 1200], f16)

            def acc_ap(bank, ci):
                return _ap(bank[ci], 0, [[86, ROWS_PER], [1, COLS]])

            def dn(tile_, ci):
                return _ap(tile_, CCOLS * ci, [[COLS, ROWS_PER], [1, COLS]])

            def emit_recip():
                for ci in range(NRCH):
                    _act_raw(nc, dn(r0_t, ci), acc_ap(accW, ci),
                             mybir.ActivationFunctionType.Reciprocal,
                             bias=float(g_center))
                    nc.vector.scalar_tensor_tensor(
                        dn(t_t, ci), acc_ap(accW, ci), float(g_center),
                        dn(r0_t, ci),
                        mybir.AluOpType.add, mybir.AluOpType.mult)
                    nc.vector.scalar_tensor_tensor(
                        dn(q_t, ci), dn(t_t, ci), 2.0, dn(r0_t, ci),
                        mybir.AluOpType.subtract, mybir.AluOpType.mult)

            def emit_tail_chunk(ci):
                # final chunk split in two so the last x/add/dma cascade is
                # half as deep
                parts = ((0, ROWS_PER),) if ci < NRCH - 1 else \
                    ((0, 3), (3, ROWS_PER))
                for (r0, r1) in parts:
                    nr = r1 - r0
                    def sl(t):
                        return _ap(t, CCOLS * ci + COLS * r0,
                                   [[COLS, nr], [1, COLS]])
                    nc.vector.scalar_tensor_tensor(
                        sl(x_t),
                        _ap(accS[ci], 86 * r0, [[86, nr], [1, COLS]]),
                        -1.0, sl(q_t),
                        mybir.AluOpType.mult, mybir.AluOpType.mult)
                    iv = _ap(at, PMIN + 86 * (ROWS_PER * ci + r0),
                             [[86, nr], [1, COLS]])
                    nc.vector.tensor_tensor(sl(of), sl(x_t), iv,
                                            mybir.AluOpType.add)
                    c0 = CCOLS * ci + COLS * r0
                    c1 = CCOLS * ci + COLS * r1
                    nc.sync.dma_start(out=o_ext[:, c0:c1],
                                      in_=of[:, c0:c1])

            def emit_W(g):
                for (vt, off, slot) in side_views(g, "W"):
                    for ci in range(NRCH):
                        mm(accW, slot, vt, off, ci, nW, W_LAST)

            def emit_S(g):
                for (vt, off, slot) in side_views(g, "S"):
                    for ci in range(NRCH):
                        mm(accS, slot, vt, off, ci, nS, S_LAST)

            g0, gS = groups[0], groups[1]
            rest = groups[2:]
            emit_sub(g0)
            emit_sub(gS, cpool)
            emit_mul(g0)
            emit_W(g0)
            emit_mul(gS, cpool)
            emit_W(gS)
            emit_sub(rest[0])
            for i, g in enumerate(rest):
                if i + 1 < len(rest):
                    emit_sub(rest[i + 1])
                emit_mul(g)
                emit_W(g)
                if i == len(rest) - 1:
                    emit_recip()
                emit_S(g)
                if i == 0:
                    emit_S(g0)
            sviews = side_views(gS, "S")
            for ci in range(NRCH):
                for (vt, off, slot) in sviews:
                    mm(accS, slot, vt, off, ci, nS, S_LAST)
                emit_tail_chunk(ci)
    nc.compile()
    return nc


def _get_nc(fast):
    assert fast, "fallback path is numpy-only"
    gc = _cache["g_center"]
    if _cache.get("fast_gc") != gc:
        _cache["fast"] = _build_fast(gc)
        _cache["fast_gc"] = gc
    return _cache["fast"]


def _shard_image(I):
    """I: (N,1,H,W) f32 -> per-core [P, AL] fp16 tiles."""
    Ip = np.zeros((N, H + 2 * PAD, W + 2 * PAD), np.float16)
    Ip[:, PAD:PAD + H, PAD:PAD + W] = I[:, 0]
    shards = []
    s0, s1 = Ip.strides[1], Ip.strides[2]
    for c in range(NCORES):
        blk = Ip[:, :, COLS * c:COLS * c + CW]       # (4, 486, 86)
        bands = np.lib.stride_tricks.as_strided(
            blk, shape=(N, 32, TR, CW),
            strides=(Ip.strides[0], BROWS * s0, s0, s1))
        shards.append(np.ascontiguousarray(bands).reshape(P, AL))
    return shards


def _eye(gdict):
    eye = np.zeros((P, 2 * NS, P), np.float32)
    idx = np.arange(P)
    for s, r2 in enumerate(_R2S):
        k = gdict[r2] * math.sqrt(math.pi) / 2.0
        eye[idx, s, idx] = k
        eye[idx, NS + s, idx] = -k
    return eye.astype(np.float16)


def _prepare(I, g):
    I = np.ascontiguousarray(np.asarray(I, dtype=np.float32))
    g = np.asarray(g, dtype=np.float32)
    gs = g[0, :, 0, 0]
    fast = bool(np.array_equal(
        g, np.broadcast_to(gs[None, :, None, None], g.shape))) and bool(
        np.all(gs > 0))
    if not fast:
        return False, None
    gdict = {}
    ok = True
    for t in range(NT):
        r2 = (t // K - PAD) ** 2 + (t % K - PAD) ** 2
        if r2 in gdict:
            ok = ok and abs(gdict[r2] - float(gs[t])) <= 1e-6 * abs(gdict[r2])
        else:
            gdict[r2] = float(gs[t])
    if not ok:
        return False, None
    _cache["g_center"] = gdict[0]
    eye = _eye(gdict)
    in_maps = [{"a": a, "eye": eye} for a in _shard_image(I)]
    return True, in_maps


def _numpy_fallback(I, g):
    I64 = np.asarray(I, np.float64)
    Ip = np.pad(I64[:, 0], ((0, 0), (PAD, PAD), (PAD, PAD)))
    out_w = np.zeros((N, H, W))
    out_s = np.zeros((N, H, W))
    g64 = np.asarray(g, np.float64)
    for t in range(NT):
        y, x = divmod(t, K)
        tap = Ip[:, y:y + H, x:x + W]
        d = tap - I64[:, 0]
        e = np.exp(-d * d / SIGMA_COLOR) * g64[:, t]
        out_w += e
        out_s += e * tap
    return (out_s / out_w).astype(np.float32)


def kernel(I, g):
    fast, in_maps = _prepare(I, g)
    if not fast:
        return _numpy_fallback(I, g)
    nc = _get_nc(True)
    try:
        res = run_bass_kernel_spmd(nc, in_maps, list(range(NCORES)))
    except Exception:
        res = run_bass_kernel_spmd(nc, in_maps, list(range(NCORES)))
    out = np.empty((N, H, W), np.float32)
    for c in range(NCORES):
        o = res.results[c]["o"]                      # [P, 1200] f16
        out[:, :, COLS * c:COLS * (c + 1)] = (
            o.reshape(N, H, COLS).astype(np.float32))
    return out


# revision 6
# speedup vs baseline: 1.0871x; 1.0095x over previous
"""Bilateral filter (K=7, sigma_color=0.1) on 8 Trainium2 NeuronCores.

Odd-symmetry band-layout formulation:
    If = I + S'/W
    S'  = sum_{pairs o} k_o * [u_o(p) - u_o(p-o)],   u_o = h_o * d_o
    W   = g_c + sum_{pairs o} k_o * [h_o(p) + h_o(p-o)]
    d_o(x) = I(x+o) - I(x),  h_o = (2/sqrt(pi)) exp(-d^2/sc) via ACT D_ERF,
    k_o = g_o * sqrt(pi)/2.
Each opposite tap pair shares ONE subtract, ONE activation, ONE multiply
over a slightly-expanded flat domain; the center tap vanishes from S' and
enters W as the reciprocal's bias.

Device mapping:
- 8 cores = 8 column bands of 80 cols (+3 halo each side -> 86).
- 128 partitions = 4 batches x 32 row-bands of 15 rows; each partition
  stores its 15 rows + 3-row halo as a flat [21*86]=1806 fp16 tile. All
  taps are pure flat-offset views; per-row "dead" halo columns flow
  through harmlessly and are skipped by the output DMA.
- Pairs with consecutive offsets Delta=86*oy+ox merge into single
  rank-2 sub / D_ERF / mul ops (stride-0 in0, stride -1 in1); ~10
  groups, sized small at the ends so the PE pipeline ramps fast and
  drains short.
- PE accumulates +-k-scaled views of h (-> W) and u (-> S') with
  scaled-identity weights into 6 SEPARATE single-bank PSUM tiles
  (banked-rows layout: 5 output rows per bank, dense 80-col stream) --
  separate tiles keep the epilogue's PSUM reads from serializing later
  matmuls. A few pair-sums of h are precombined on DVE (NCONV_W) to
  shave PE columns. W finishes before the last S views so the
  reciprocal+Newton overlap the S tail; the output leaves as fp16.
"""
import math

import numpy as np

import concourse.bacc as bacc
import concourse.tile as tile
from concourse import mybir
from concourse.bass_utils import run_bass_kernel_spmd
import bass_rust

K = 7
PAD = K // 2
H, W = 480, 640
N = 4
NCORES = 8
SIGMA_COLOR = 2.0 * 0.1 ** 2            # 0.02
CSC = 1.0 / math.sqrt(SIGMA_COLOR)      # DErf(d*CSC) = 2/sqrt(pi)*exp(-d^2/sc)
NT = K * K

P = 128                                  # partitions (4 batches x 32 bands)
BROWS = 15                               # output rows per partition
TR = BROWS + 2 * PAD                     # 21 stored rows
COLS = W // NCORES                       # 80 output cols per core
CW = COLS + 2 * PAD                      # 86 stored cols
AL = TR * CW                             # 1806 flat image elems
PMIN = PAD * CW + PAD                    # 261: flat index of first output px
FL = (BROWS - 1) * CW + COLS             # 1284: flat output span (w/ dead)
SB = 1536                                # S'-half offset in PSUM (bank 3)

f32 = mybir.dt.float32
f16 = mybir.dt.float16

# canonical pairs grouped by oy-row: (oy, [ox...]), Delta = 86*oy + ox
_ROWS = [(0, [1, 2, 3]), (1, list(range(-3, 4))),
         (2, list(range(-3, 4))), (3, list(range(-3, 4)))]
_R2S = sorted({oy * oy + ox * ox for oy, oxs in _ROWS for ox in oxs})
NS = len(_R2S)                           # 9 radius classes

NCONV_W = 10
NCONV_S = 0

_cache = {}


def _ap(base, off, dims):
    """Rank-(1+len(dims)) AP on base's tile at element offset off."""
    return bass_rust.AP(base.tensor, base.offset + off,
                        [list(base.ap[0])] + [list(d) for d in dims])


def _act_raw(nc, out, in_, func, bias=0.0, scale=1.0):
    """Raw InstActivation (bass blocks Reciprocal in the wrapper; a Newton
    step at the call site restores accuracy)."""
    eng = nc.scalar
    inputs = [eng.lower_ap(in_)]
    for arg in (bias, scale, 0.0):
        inputs.append(mybir.ImmediateValue(dtype=mybir.dt.float32, value=arg))
    return eng.add_instruction(mybir.InstActivation(
        name=nc.get_next_instruction_name(), func=func,
        ins=inputs, outs=[eng.lower_ap(out)]))


def _build_fast(g_center):
    nc = bacc.Bacc("TRN2", target_bir_lowering=False, debug=False,
                   num_devices=NCORES)
    a_ext = nc.declare_dram_parameter("a", [P, AL], f16, isOutput=False)
    eye_ext = nc.declare_dram_parameter("eye", [P, 2 * NS, P], f16,
                                        isOutput=False)
    o_ext = nc.declare_dram_parameter("o", [P, 1200], f16, isOutput=True)

    # PSUM banked-rows layout: bank b holds output rows 5b..5b+4 as a
    # 5x86-flat span (430 fp32 of 512); W in banks 0-2, S' in banks 3-5.
    NRCH = 3
    ROWS_PER = 5
    CCOLS = ROWS_PER * COLS                       # 400 dense cols per chunk

    gspecs = [(0, [1]), (0, [2, 3]),
              (1, [-3, -2]), (1, [-1, 0]), (1, [1, 2, 3]),
              (2, [-3, -2, -1, 0]), (2, [1, 2, 3]),
              (3, [-3, -2, -1, 0]), (3, [1]), (3, [2, 3])]
    groups = []
    for oy, ss in gspecs:
        dmin, dmax = 86 * oy + ss[0], 86 * oy + ss[-1]
        groups.append(dict(oy=oy, oxs=ss, np=len(ss), dmin=dmin,
                           lbar=FL + dmax))
    NPAIRS = sum(g["np"] for g in groups)

    # W-side presum conversions (flat adds on DVE; views stay banked);
    # picked from middle groups so neither PE start nor tail blocks on them
    conv_w = set()
    for g in groups[4:9]:
        for j in list(range(g["np"]))[:2]:
            if len(conv_w) < NCONV_W:
                conv_w.add((id(g), j))
    W_LAST = 2 * NPAIRS - len(conv_w)
    S_LAST = 2 * NPAIRS

    with tile.TileContext(nc, pool_alloc_mode="queue") as tc:
        with tc.tile_pool(name="work", bufs=3) as pool, \
             tc.tile_pool(name="cst", bufs=1) as cpool, \
             tc.tile_pool(name="ep", bufs=1) as epool, \
             tc.tile_pool(name="ps", bufs=1, space="PSUM") as ppool:
            at = cpool.tile([P, AL], f16)
            eye_t = cpool.tile([P, 2 * NS, P], f16)
            # HAM warmup: PE boots throttled to 1.2 GHz and un-throttles
            # only after ~10us of sustained activity. Burn the idle
            # preamble window with garbage matmuls (uninitialized SBUF ->
            # scratch bank) so the real stream starts warm.
            wt = cpool.tile([P, 704], f16)
            nc.gpsimd.memset(wt, 0.0)
            wacc = ppool.tile([P, 512], f32, name="wacc")
            for _ in range(14):
                nc.tensor.matmul(wacc[:, 0:512], wt[:, 0:128],
                                 wt[:, 128:640], start=True, stop=True)
            nc.sync.dma_start(out=at[:, 172:1634], in_=a_ext[:, 172:1634])
            nc.gpsimd.dma_start(out=at[:, 0:172], in_=a_ext[:, 0:172])
            nc.gpsimd.dma_start(out=at[:, 1634:AL], in_=a_ext[:, 1634:AL])
            nc.sync.dma_start(out=eye_t[:, 0:NS, :], in_=eye_ext[:, 0:NS, :])
            nc.gpsimd.dma_start(out=eye_t[:, NS:2 * NS, :],
                                in_=eye_ext[:, NS:2 * NS, :])

            accW = [ppool.tile([P, 512], f32, name=f"aw{i}")
                    for i in range(3)]
            accS = [ppool.tile([P, 512], f32, name=f"as{i}")
                    for i in range(3)]

            nW = [0, 0, 0]
            nS = [0, 0, 0]

            def emit_sub(g, po=None):
                po = po or pool
                np_, lbar, dmin = g["np"], g["lbar"], g["dmin"]
                oy, ox0 = g["oy"], g["oxs"][0]
                nm = f"{oy}_{ox0}"
                dt = po.tile([P, np_, lbar], f16, name=f"d{nm}", tag="d")
                ht = po.tile([P, np_, lbar], f16, name=f"h{nm}", tag="h")
                in0 = _ap(at, PMIN, [[0, np_], [1, lbar]])
                in1 = _ap(at, PMIN - dmin, [[-1, np_], [1, lbar]])
                do = _ap(dt, 0, [[lbar, np_], [1, lbar]])
                nc.vector.tensor_tensor(do, in0, in1,
                                        mybir.AluOpType.subtract)
                nc.scalar.activation(ht[:, :, :], dt[:, :, :],
                                     mybir.ActivationFunctionType.
                                     Derivative_Erf, bias=0.0, scale=CSC)
                g["dt"], g["ht"] = dt, ht

            def emit_mul(g, po=None):
                po = po or pool
                np_, lbar = g["np"], g["lbar"]
                oy, ox0 = g["oy"], g["oxs"][0]
                dt, ht = g["dt"], g["ht"]
                ut = po.tile([P, np_, lbar], f16, name=f"u{oy}_{ox0}",
                             tag="u")
                nc.vector.tensor_tensor(ut[:, :, :], ht[:, :, :],
                                        dt[:, :, :], mybir.AluOpType.mult)
                g["ut"] = ut
                tfh = ht.rearrange("p a b -> p (a b)")
                g["hs"] = {}
                for j, ox in enumerate(g["oxs"]):
                    if (id(g), j) not in conv_w:
                        continue
                    delta = 86 * oy + ox
                    ps_t = pool.tile([P, FL], f16,
                                     name=f"c{oy}_{ox}", tag="c")
                    nc.vector.tensor_tensor(
                        ps_t[:, :],
                        _ap(tfh, j * lbar + delta, [[1, FL]]),
                        _ap(tfh, j * lbar, [[1, FL]]),
                        mybir.AluOpType.add)
                    g["hs"][j] = ps_t

            def mm(bank, slot, vt, off, ci, nn, last):
                rhs = _ap(vt, off + 86 * ROWS_PER * ci,
                          [[86, ROWS_PER], [1, COLS]])
                out = _ap(bank[ci], 0, [[86, ROWS_PER], [1, COLS]])
                nc.tensor.matmul(out, eye_t[:, slot, :], rhs,
                                 start=(nn[ci] == 0),
                                 stop=(nn[ci] == last - 1))
                nn[ci] += 1

            def side_views(g, side):
                oy, lbar = g["oy"], g["lbar"]
                t = g["ht"] if side == "W" else g["ut"]
                tf = t.rearrange("p a b -> p (a b)")
                views = []
                pres = []
                for j, ox in enumerate(g["oxs"]):
                    delta = 86 * oy + ox
                    s = _R2S.index(oy * oy + ox * ox)
                    if side == "W" and j in g["hs"]:
                        pres.append((g["hs"][j], 0, s))
                    else:
                        views.append((tf, j * lbar + delta, s))
                        views.append((tf, j * lbar,
                                      s if side == "W" else NS + s))
                return views + pres

            # epilogue tiles (dense 1200)
            r0_t = epool.tile([P, 1200], f32)
            t_t = epool.tile([P, 1200], f32)
            q_t = epool.tile([P, 1200], f32)
            of = epool.tile([P, 1200], f16)

            def acc_ap(bank, ci):
                return _ap(bank[ci], 0, [[86, ROWS_PER], [1, COLS]])

            def dn(tile_, ci):
                return _ap(tile_, CCOLS * ci, [[COLS, ROWS_PER], [1, COLS]])

            def emit_recip():
                for ci in range(NRCH):
                    _act_raw(nc, dn(r0_t, ci), acc_ap(accW, ci),
                             mybir.ActivationFunctionType.Reciprocal,
                             bias=float(g_center))
                    nc.vector.scalar_tensor_tensor(
                        dn(t_t, ci), acc_ap(accW, ci), float(g_center),
                        dn(r0_t, ci),
                        mybir.AluOpType.add, mybir.AluOpType.mult)
                    nc.vector.scalar_tensor_tensor(
                        dn(q_t, ci), dn(t_t, ci), 2.0, dn(r0_t, ci),
                        mybir.AluOpType.subtract, mybir.AluOpType.mult)

            def emit_tail_chunk(ci):
                # final chunk split in two so the last x/add/dma cascade is
                # half as deep
                parts = ((0, ROWS_PER),) if ci < NRCH - 1 else \
                    ((0, 3), (3, ROWS_PER))
                for (r0, r1) in parts:
                    nr = r1 - r0
                    def sl(t):
                        return _ap(t, CCOLS * ci + COLS * r0,
                                   [[COLS, nr], [1, COLS]])
                    # of = S'/W = (-S')*q, q = -1/W; host adds I back
                    nc.vector.scalar_tensor_tensor(
                        sl(of),
                        _ap(accS[ci], 86 * r0, [[86, nr], [1, COLS]]),
                        -1.0, sl(q_t),
                        mybir.AluOpType.mult, mybir.AluOpType.mult)
                    c0 = CCOLS * ci + COLS * r0
                    c1 = CCOLS * ci + COLS * r1
                    nc.sync.dma_start(out=o_ext[:, c0:c1],
                                      in_=of[:, c0:c1])

            def emit_W(g):
                for (vt, off, slot) in side_views(g, "W"):
                    for ci in range(NRCH):
                        mm(accW, slot, vt, off, ci, nW, W_LAST)

            def emit_S(g):
                for (vt, off, slot) in side_views(g, "S"):
                    for ci in range(NRCH):
                        mm(accS, slot, vt, off, ci, nS, S_LAST)

            g0, gS = groups[0], groups[1]
            rest = groups[2:]
            emit_sub(g0)
            emit_sub(gS, cpool)
            emit_mul(g0)
            emit_W(g0)
            emit_mul(gS, cpool)
            emit_W(gS)
            emit_sub(rest[0])
            for i, g in enumerate(rest):
                if i + 1 < len(rest):
                    emit_sub(rest[i + 1])
                emit_mul(g)
                emit_W(g)
                if i == len(rest) - 1:
                    emit_recip()
                emit_S(g)
                if i == 0:
                    emit_S(g0)
            sviews = side_views(gS, "S")
            for ci in range(NRCH):
                for (vt, off, slot) in sviews:
                    mm(accS, slot, vt, off, ci, nS, S_LAST)
                emit_tail_chunk(ci)
    nc.compile()
    return nc


def _get_nc(fast):
    assert fast, "fallback path is numpy-only"
    gc = _cache["g_center"]
    if _cache.get("fast_gc") != gc:
        _cache["fast"] = _build_fast(gc)
        _cache["fast_gc"] = gc
    return _cache["fast"]


def _shard_image(I):
    """I: (N,1,H,W) f32 -> per-core [P, AL] fp16 tiles."""
    Ip = np.zeros((N, H + 2 * PAD, W + 2 * PAD), np.float16)
    Ip[:, PAD:PAD + H, PAD:PAD + W] = I[:, 0]
    shards = []
    s0, s1 = Ip.strides[1], Ip.strides[2]
    for c in range(NCORES):
        blk = Ip[:, :, COLS * c:COLS * c + CW]       # (4, 486, 86)
        bands = np.lib.stride_tricks.as_strided(
            blk, shape=(N, 32, TR, CW),
            strides=(Ip.strides[0], BROWS * s0, s0, s1))
        shards.append(np.ascontiguousarray(bands).reshape(P, AL))
    return shards


def _eye(gdict):
    eye = np.zeros((P, 2 * NS, P), np.float32)
    idx = np.arange(P)
    for s, r2 in enumerate(_R2S):
        k = gdict[r2] * math.sqrt(math.pi) / 2.0
        eye[idx, s, idx] = k
        eye[idx, NS + s, idx] = -k
    return eye.astype(np.float16)


def _prepare(I, g):
    I = np.ascontiguousarray(np.asarray(I, dtype=np.float32))
    g = np.asarray(g, dtype=np.float32)
    gs = g[0, :, 0, 0]
    fast = bool(np.array_equal(
        g, np.broadcast_to(gs[None, :, None, None], g.shape))) and bool(
        np.all(gs > 0))
    if not fast:
        return False, None
    gdict = {}
    ok = True
    for t in range(NT):
        r2 = (t // K - PAD) ** 2 + (t % K - PAD) ** 2
        if r2 in gdict:
            ok = ok and abs(gdict[r2] - float(gs[t])) <= 1e-6 * abs(gdict[r2])
        else:
            gdict[r2] = float(gs[t])
    if not ok:
        return False, None
    _cache["g_center"] = gdict[0]
    eye = _eye(gdict)
    in_maps = [{"a": a, "eye": eye} for a in _shard_image(I)]
    return True, in_maps


def _numpy_fallback(I, g):
    I64 = np.asarray(I, np.float64)
    Ip = np.pad(I64[:, 0], ((0, 0), (PAD, PAD), (PAD, PAD)))
    out_w = np.zeros((N, H, W))
    out_s = np.zeros((N, H, W))
    g64 = np.asarray(g, np.float64)
    for t in range(NT):
        y, x = divmod(t, K)
        tap = Ip[:, y:y + H, x:x + W]
        d = tap - I64[:, 0]
        e = np.exp(-d * d / SIGMA_COLOR) * g64[:, t]
        out_w += e
        out_s += e * tap
    return (out_s / out_w).astype(np.float32)


def kernel(I, g):
    fast, in_maps = _prepare(I, g)
    if not fast:
        return _numpy_fallback(I, g)
    nc = _get_nc(True)
    try:
        res = run_bass_kernel_spmd(nc, in_maps, list(range(NCORES)))
    except Exception:
        res = run_bass_kernel_spmd(nc, in_maps, list(range(NCORES)))
    out = np.empty((N, H, W), np.float32)
    I32 = np.asarray(I, dtype=np.float32)
    for c in range(NCORES):
        o = res.results[c]["o"]                      # [P, 1200] f16: S'/W
        out[:, :, COLS * c:COLS * (c + 1)] = (
            o.reshape(N, H, COLS).astype(np.float32)
            + I32[:, 0, :, COLS * c:COLS * (c + 1)])
    return out
